# revision 1
# baseline (speedup 1.0000x reference)
"""Trainium2 Bass kernel: row-wise Linear(64->64) + LayerNorm + LeakyReLU(0.2).

Math: out = leaky_relu(layernorm(x @ W.T + b) * gamma + beta), row-independent.
`batch` does not affect the computation (layernorm is per-row).

Device strategy (per core, data-parallel over 8 cores):
  - Host packs the core's row shard [Nc, 64] f32 into a feature-major layout
    xh [128, C]: partitions = (block b in {0,1}) * 64 + feature f, free = C
    columns, one column per row index within the block.  Two row-blocks are
    stacked on the partition dim so every DMA and matmul uses all 128
    partitions.
  - Host centers the weights: Wc = W.T - colmean(W.T), bc = b - mean(b), so
    the matmul directly produces s = y - mean(y) (mean over out features).
    W is applied as a block-diagonal [128, 128] (one 64x64 block per row
    block); each matmul tile lhsT = xh[:, t*128:(t+1)*128] (stationary),
    rhs = Wblk, giving PSUM out [128 rows, 2 groups x 64 feats].
  - bc is added with one extra K=2 bf16 matmul (ones lhsT; rhs rows are the
    bf16 hi/lo split of bc, so the bias is fp32-exact to ~2^-18).
  - Per PSUM quad (4 tiles, one 2KB bank): one DVE bn_stats gives per-group
    even/odd (count, mean, count*var); chunk-level DVE ops combine them into
    var, add eps, and compute inv = rsqrt(var+eps) via the int32 bit-trick
    seed + 2 Newton iterations (no ScalarE table switches).
  - Normalize+activation: out = Lrelu(s * inv) per (tile, group), fused in
    one ScalarE activation op (scale = per-partition inv, alpha = 0.2);
    optionally some groups are routed to DVE (tensor_scalar mul + max(a*t,t))
    to balance engine load.
  - gamma/beta are ones/zeros in this problem; a host fallback handles the
    general case.
"""

import os
import sys
import numpy as np
import ml_dtypes

import concourse.bass as bass
import concourse.bacc as bacc
import concourse.tile as tile
from concourse import mybir
from concourse.bass_utils import run_bass_kernel_spmd

F32 = mybir.dt.float32
BF16 = mybir.dt.bfloat16
I32 = mybir.dt.int32
AX = mybir.AluOpType
AF = mybir.ActivationFunctionType

IN_F = 64
OUT_F = 64
EPS = 1e-5
ALPHA = 0.2
N_CORES = 8
N_NODES = 2_000_000

RSQRT_MAGIC = 0x5F375A86

# --- tunables -------------------------------------------------------------
CHUNK_COLS = 4096          # columns (row-indices per block) per chunk
ACT_GROUPS = 8             # of the 8 (tile, group) normalizes per quad, how
                           # many run on ScalarE (rest on VectorE)
IN_BUFS = 3
OUT_BUFS = 3
PSUM_BUFS = 8
DMA_ENGINE = "sync"        # engine issuing chunk DMAs
LEAKY_ENGINE = "vector"    # "vector" or "gpsimd"


def _dma(nc):
    return getattr(nc, DMA_ENGINE)


def build_module(cols, chunk_cols=None, act_groups=None, passes=1,
                 in_bufs=None, out_bufs=None, psum_bufs=None,
                 leaky_engine=None, dma_engine=None, variant="full",
                 newton_iters=2, dyn_reps=False, store_engine=None):
    """Build and compile the Bass module for a per-core shard with `cols`
    columns per block (cols*2 row-instances).  cols % 128 == 0.
    passes>1 repeats the whole computation (for differential timing)."""
    chunk_cols = CHUNK_COLS if chunk_cols is None else chunk_cols
    act_groups = ACT_GROUPS if act_groups is None else act_groups
    in_bufs = IN_BUFS if in_bufs is None else in_bufs
    out_bufs = OUT_BUFS if out_bufs is None else out_bufs
    psum_bufs = PSUM_BUFS if psum_bufs is None else psum_bufs
    leaky_engine = LEAKY_ENGINE if leaky_engine is None else leaky_engine
    dma_engine = DMA_ENGINE if dma_engine is None else dma_engine
    store_engine = dma_engine if store_engine is None else store_engine
    assert cols % 128 == 0
    nc = bacc.Bacc(
        "TRN2", target_bir_lowering=False, debug=False, enable_asserts=False
    )
    xh = nc.dram_tensor("xh", [128, cols], F32, kind="ExternalInput").ap()
    if dyn_reps:
        reps = nc.dram_tensor("reps", [1, 1], I32, kind="ExternalInput").ap()
    wblk = nc.dram_tensor("wblk", [128, 128], F32, kind="ExternalInput").ap()
    onesw = nc.dram_tensor("onesw", [2, 128], BF16, kind="ExternalInput").ap()
    bq = nc.dram_tensor("bq", [2, 512], BF16, kind="ExternalInput").ap()
    zh = nc.dram_tensor("zh", [128, cols], F32, kind="ExternalOutput").ap()

    # chunk layout
    chunks = []
    c0 = 0
    while c0 < cols:
        fc = min(chunk_cols, cols - c0)
        chunks.append((c0, fc))
        c0 += fc

    with tile.TileContext(nc) as tc:
        with (
            tc.tile_pool(name="const", bufs=1) as constp,
            tc.tile_pool(name="inp", bufs=in_bufs) as inp,
            tc.tile_pool(name="outp", bufs=out_bufs) as outp,
            tc.tile_pool(name="psump", bufs=psum_bufs, space="PSUM") as psump,
            tc.tile_pool(name="statsp", bufs=2) as statsp,
            tc.tile_pool(name="miscp", bufs=2) as miscp,
        ):
            wblk_sb = constp.tile([128, 128], F32, name="wblk_sb")
            nc.sync.dma_start(wblk_sb[:, :], wblk)
            ones_sb = constp.tile([2, 128], BF16, name="ones_sb")
            nc.sync.dma_start(ones_sb[:, :], onesw)
            bq_sb = constp.tile([2, 512], BF16, name="bq_sb")
            nc.sync.dma_start(bq_sb[:, :], bq)

            import contextlib
            if dyn_reps:
                reps_sb = constp.tile([1, 1], I32, name="reps_sb")
                nc.sync.dma_start(reps_sb[:, :], reps)
                rv = nc.values_load(reps_sb[0:1, 0:1], min_val=0, max_val=64, skip_runtime_bounds_check=True)
                loop_cm = tc.For_i(0, rv, 1)
            else:
                loop_cm = contextlib.nullcontext()
            with loop_cm:
              for ci, (c0, fc) in enumerate(chunks * passes):
                  ntiles = fc // 128
                  G = ntiles * 2
                  nquads = (ntiles + 3) // 4

                  xin = inp.tile([128, chunk_cols], F32, name="xin", tag="xin")
                  getattr(nc, dma_engine).dma_start(xin[:, 0:fc], xh[:, c0 : c0 + fc])
                  zout = outp.tile([128, chunk_cols], F32, name="zout", tag="zout")

                  if variant == "memcpy":
                      _dma(nc).dma_start(zh[:, c0 : c0 + fc], xin[:, 0:fc])
                      continue

                  # PSUM columns are group-interleaved (col 2*o+g holds group
                  # g's feature o), so bn_stats' even/odd split is exactly the
                  # per-group split: 6-tuple = (64, mean_g0, 64*var_g0,
                  #                             64, mean_g1, 64*var_g1).
                  stats = statsp.tile([128, ntiles, 6], F32, name="stats",
                                      tag="stats",
                                      padded_shape=[128, chunk_cols // 128, 6])

                  ps_list = []
                  for q in range(nquads):
                      tq = min(4, ntiles - q * 4)
                      nq = tq * 128
                      ps = psump.tile([128, 512], F32, name="ps", tag="ps")
                      ps_list.append((ps, tq))
                      # one well-formed accumulation group per quad: bias first
                      # (start=True over the whole region), mains accumulate
                      nc.tensor.matmul(
                          ps[:, 0:nq],
                          ones_sb[:, :],
                          bq_sb[:, 0:nq],
                          start=True,
                          stop=False,
                          skip_group_check=True,
                      )
                      for t in range(tq):
                          gt = q * 4 + t
                          nc.tensor.matmul(
                              ps[:, t * 128 : (t + 1) * 128],
                              xin[:, gt * 128 : (gt + 1) * 128],
                              wblk_sb[:, :],
                              start=False,
                              stop=(t == tq - 1),
                              skip_group_check=True,
                          )
                      if variant not in ("nostats", "nonorm"):
                          for t in range(tq):
                              gt = q * 4 + t
                              nc.vector.bn_stats(
                                  stats[:, gt, :],
                                  ps[:, t * 128 : (t + 1) * 128],
                              )

                  # ---- chunk-level: inv = rsqrt(var + eps)
                  skip_stats = variant in ("nostats", "nonorm")
                  ve = miscp.tile([128, G], F32, name="ve", tag="ve",
                                  padded_shape=[128, chunk_cols // 64])
                  # ve = (cnt*var)/64 + eps; cnt*var slots are 2 and 5
                  if not skip_stats:
                      nc.vector.tensor_scalar(
                          ve[:, :], stats[:, :, 2::3], 1.0 / 64.0, float(EPS),
                          op0=AX.mult, op1=AX.add,
                      )
                  # u0 = bitcast(MAGIC - (bitcast_i32(ve) >> 1))
                  u = miscp.tile([128, G], F32, name="u", tag="u",
                                 padded_shape=[128, chunk_cols // 64])
                  if not skip_stats:
                      ui = u.bitcast(I32)
                      nc.vector.tensor_scalar(
                          ui[:, :], ve.bitcast(I32)[:, :], 1, None,
                          op0=AX.logical_shift_right,
                      )
                      # MAGIC - t == (t ^ -1) + (MAGIC + 1); bitwise and arith
                      # ops cannot be mixed in one tensor_scalar.
                      nc.vector.tensor_scalar(
                          ui[:, :], ui[:, :], -1, None, op0=AX.bitwise_xor
                      )
                      nc.vector.tensor_scalar(
                          ui[:, :], ui[:, :], RSQRT_MAGIC + 1, None, op0=AX.add
                      )
                      # 2 Newton iterations: u = u * (1.5 - 0.5 * ve * u^2)
                      t1 = miscp.tile([128, G], F32, name="t1", tag="t1",
                                      padded_shape=[128, chunk_cols // 64])
                      t2 = miscp.tile([128, G], F32, name="t2", tag="t2",
                                      padded_shape=[128, chunk_cols // 64])
                      for _ in range(newton_iters):
                          nc.vector.tensor_tensor(t1[:, :], u[:, :], u[:, :], op=AX.mult)
                          nc.vector.scalar_tensor_tensor(
                              t2[:, :], ve[:, :], -0.5, t1[:, :],
                              op0=AX.mult, op1=AX.mult,
                          )
                          nc.vector.scalar_tensor_tensor(
                              u[:, :], t2[:, :], 1.5, u[:, :],
                              op0=AX.add, op1=AX.mult,
                          )
                  inv = u  # [128, G] = per (tile, group) rsqrt(var+eps)

                  # ---- normalize + leaky relu
                  for q in range(nquads):
                      ps, tq = ps_list[q]
                      for t in range(tq):
                          gt = q * 4 + t
                          psv = ps[:, t * 128 : (t + 1) * 128].rearrange(
                              "p (o g) -> p g o", g=2
                          )
                          for g in range(2):
                              ocol = gt * 128 + g * 64
                              sl = (1.0 if skip_stats else
                                    inv[:, gt * 2 + g : gt * 2 + g + 1])
                              # t_g = s_g * inv_g  (leaky applied afterwards;
                              # valid because inv > 0 commutes with leaky)
                              if (t * 2 + g) < act_groups:
                                  nc.scalar.activation(
                                      zout[:, ocol : ocol + 64],
                                      psv[:, g, :],
                                      AF.Copy,
                                      bias=0.0,
                                      scale=sl,
                                  )
                              else:
                                  nc.vector.tensor_scalar(
                                      zout[:, ocol : ocol + 64],
                                      psv[:, g, :], sl, None,
                                      op0=AX.mult,
                                  )
                  # leaky relu in place over the whole chunk's output
                  if variant not in ("nonorm", "noleaky"):
                      zc = zout[:, 0:fc]
                      nc.vector.scalar_tensor_tensor(
                          zc, zc, ALPHA, zc, op0=AX.mult, op1=AX.max
                      )

                  getattr(nc, store_engine).dma_start(zh[:, c0 : c0 + fc], zout[:, 0:fc])

    nc.compile()
    return nc


# ---------------------------------------------------------------------------
# host-side packing / unpacking
# ---------------------------------------------------------------------------

def _pack_core(shard, cols):
    """[rows, 64] f32 -> xh [128, cols] f32 (two stacked feature-major blocks)."""
    rows = shard.shape[0]
    assert rows % 2 == 0
    half = rows // 2
    ntile = cols // 128
    xpad = np.zeros((2 * cols, 64), dtype=np.float32)
    xpad[:half] = shard[:half]
    xpad[cols : cols + half] = shard[half:]
    # xh[b*64+f, T*128+m] = xpad[b*cols + T*128 + m, f]
    xh = (
        xpad.reshape(2, ntile, 128, 64)
        .transpose(0, 3, 1, 2)
        .reshape(128, cols)
    )
    return np.ascontiguousarray(xh)


def _unpack_core(zh, cols, rows):
    """zh [128, cols] f32 -> [rows, 64] f32.

    zh[m, T*128 + g*64 + o] = z[g*cols + T*128 + m, o]"""
    half = rows // 2
    ntile = cols // 128
    zz = (
        zh.reshape(128, ntile, 2, 64)
        .transpose(2, 1, 0, 3)
        .reshape(2 * cols, 64)
    )
    return np.concatenate([zz[:half], zz[cols : cols + half]], axis=0)


def _make_weights(W, b):
    Wt = W.astype(np.float64).T  # [in_f, out_f]
    Wc = (Wt - Wt.mean(axis=1, keepdims=True)).astype(np.float32)
    # PSUM column 2*o+g holds group g's output feature o (group-interleaved)
    wblk = np.zeros((128, 128), dtype=np.float32)
    wblk[:64, 0::2] = Wc
    wblk[64:, 1::2] = Wc
    bc = (b.astype(np.float64) - b.astype(np.float64).mean()).astype(np.float32)
    bc_hi = bc.astype(ml_dtypes.bfloat16)
    bc_lo = (bc - bc_hi.astype(np.float32)).astype(ml_dtypes.bfloat16)
    bqa = np.zeros((2, 512), dtype=ml_dtypes.bfloat16)
    bqa[0] = np.tile(np.repeat(bc_hi, 2), 4)
    bqa[1] = np.tile(np.repeat(bc_lo, 2), 4)
    onesw = np.ones((2, 128), dtype=ml_dtypes.bfloat16)
    return wblk, onesw, bqa


_NC_CACHE = {}


def _get_module(cols):
    key = (cols, CHUNK_COLS, ACT_GROUPS)
    if key not in _NC_CACHE:
        _NC_CACHE[key] = build_module(cols)
    return _NC_CACHE[key]


def _host_reference(input_x, W, b, gamma, beta):
    y = input_x.astype(np.float32) @ W.T.astype(np.float32) + b
    mu = y.mean(axis=-1, keepdims=True)
    var = np.square(y - mu).mean(axis=-1, keepdims=True)
    y = (y - mu) / np.sqrt(var + EPS) * gamma + beta
    return np.where(y >= 0, y, np.float32(ALPHA) * y).astype(np.float32)


def _make_in_maps(input_x, W, b):
    n = input_x.shape[0]
    per_core = (n + N_CORES - 1) // N_CORES
    per_core += (-per_core) % 2
    half = per_core // 2
    cols = ((half + 127) // 128) * 128
    wblk, onesw, bqa = _make_weights(W, b)
    in_maps = []
    shards = []
    for i in range(N_CORES):
        lo = min(i * per_core, n)
        hi = min(lo + per_core, n)
        shard = input_x[lo:hi]
        if shard.shape[0] < per_core:
            shard = np.concatenate(
                [shard, np.zeros((per_core - shard.shape[0], IN_F), np.float32)]
            )
        shards.append((lo, hi))
        in_maps.append(
            {"xh": _pack_core(shard, cols), "wblk": wblk, "onesw": onesw,
             "bq": bqa}
        )
    return in_maps, shards, cols, per_core


def make_timed_runner(inputs, warmup=2):
    """Build a persistent sharded-jit over the 8 cores with device-resident
    inputs; returns a callable(iters) -> mean wall seconds per execution."""
    import time
    import jax
    from jax.sharding import Mesh, PartitionSpec, NamedSharding
    from jax.experimental.shard_map import shard_map
    from concourse import bass2jax, mybir as _mb

    bass2jax.install_neuronx_cc_hook()
    input_x = np.asarray(inputs["input_x"], dtype=np.float32)
    W = np.asarray(inputs["W"], dtype=np.float32)
    b = np.asarray(inputs["b"], dtype=np.float32)
    in_maps, shards, cols, per_core = _make_in_maps(input_x, W, b)
    nc = _get_module(cols)

    partition_name = (
        nc.partition_id_tensor.name if nc.partition_id_tensor else None
    )
    in_names, out_names, out_avals, zero_outs = [], [], [], []
    for alloc in nc.m.functions[0].allocations:
        if not isinstance(alloc, _mb.MemoryLocationSet):
            continue
        name = alloc.memorylocations[0].name
        if alloc.kind == "ExternalInput":
            if name != partition_name:
                in_names.append(name)
        elif alloc.kind == "ExternalOutput":
            out_names.append(name)
            shape = tuple(alloc.tensor_shape)
            dtype = _mb.dt.np(alloc.dtype)
            out_avals.append(jax.core.ShapedArray(shape, dtype))
            zero_outs.append(np.zeros(shape, dtype))
    n_params = len(in_names)
    all_names = in_names + out_names
    if partition_name is not None:
        all_names = all_names + [partition_name]

    def _body(*args):
        operands = list(args)
        if partition_name is not None:
            operands.append(bass2jax.partition_id_tensor())
        outs = bass2jax._bass_exec_p.bind(
            *operands,
            out_avals=tuple(out_avals),
            in_names=tuple(all_names),
            out_names=tuple(out_names),
            lowering_input_output_aliases=(),
            sim_require_finite=True,
            sim_require_nnan=True,
            nc=nc,
        )
        return tuple(outs)

    devices = jax.devices()[:N_CORES]
    mesh = Mesh(np.asarray(devices), ("core",))
    spec = PartitionSpec("core")
    sharded = jax.jit(
        shard_map(
            _body, mesh=mesh,
            in_specs=(spec,) * (n_params + len(out_names)),
            out_specs=(spec,) * len(out_names),
            check_rep=False,
        ),
        keep_unused=True,
    )
    sh = NamedSharding(mesh, spec)
    dev_args = [
        jax.device_put(
            np.concatenate([in_maps[c][nm] for c in range(N_CORES)], axis=0), sh
        )
        for nm in in_names
    ] + [
        jax.device_put(
            np.zeros((N_CORES * z.shape[0], *z.shape[1:]), z.dtype), sh
        )
        for z in zero_outs
    ]

    def run(iters=5):
        for _ in range(warmup):
            r = sharded(*dev_args)
            jax.block_until_ready(r)
        t0 = time.perf_counter()
        for _ in range(iters):
            r = sharded(*dev_args)
        jax.block_until_ready(r)
        return (time.perf_counter() - t0) / iters

    return run


def kernel(input_x, W, b, gamma, beta, batch=None, **_unused):
    input_x = np.asarray(input_x, dtype=np.float32)
    W = np.asarray(W, dtype=np.float32)
    b = np.asarray(b, dtype=np.float32)
    gamma = np.asarray(gamma, dtype=np.float32)
    beta = np.asarray(beta, dtype=np.float32)

    if not (np.all(gamma == 1.0) and np.all(beta == 0.0)):
        return _host_reference(input_x, W, b, gamma, beta)

    n = input_x.shape[0]
    in_maps, shards, cols, per_core = _make_in_maps(input_x, W, b)
    nc = _get_module(cols)
    res = run_bass_kernel_spmd(nc, in_maps, core_ids=list(range(N_CORES)))

    out = np.empty((n, OUT_F), dtype=np.float32)
    for i, (lo, hi) in enumerate(shards):
        zh = np.asarray(res.results[i]["zh"])
        z = _unpack_core(zh, cols, per_core)
        out[lo:hi] = z[: hi - lo]
    return out



# revision 5
# speedup vs baseline: 1.0732x; 1.0732x over previous
"""Trainium2 Bass kernel: row-wise Linear(64->64) + LayerNorm + LeakyReLU(0.2).

Math: out = leaky_relu(layernorm(x @ W.T + b)), row-independent; `batch` does
not affect the computation (layernorm is per-row).

v2 design — feature-major layout, bf16 data path, full-width ops only:

  - Host packs each core's row shard [Nc, 64] into xh [128, cols] bf16:
    partition p = (block b in {0,1})*64 + in-feature f; column c = node index
    within the block.  Two node-blocks stack on the partition dim so every
    DMA / matmul / DVE / ACT op uses all 128 partitions.
  - Weights are centered on host (Wc = W.T - rowmean over out-features,
    bc = b - mean(b)) so the matmul directly yields y = out-centered rows:
    LayerNorm's mean subtraction is free.
  - Per 512-col PSUM bank: y = Wblk.T @ x (block-diag Wc, bf16) accumulated
    with a K=2 bias matmul (bc as bf16 hi+lo rows, ones rhs).
  - ACT Square: sq = y^2 (PSUM->SBUF bf16), one full-width op.
  - PE: v = Rdiv.T @ sq with Rdiv = block-diag ones/64 -> v[q, n] = var of
    node n's block, replicated across that block's 64 partitions.
  - ACT Abs_reciprocal_sqrt: inv = (var + eps)^-1/2 (PSUM->SBUF bf16).
  - DVE: l = max(alpha*y, y) (leaky first; valid since inv > 0 commutes),
    then z = l * inv (bf16 tensor_tensor, 2x mode).
  - z streams out as bf16; host unpacks/casts to fp32.

All elementwise work is FD>=512 full-width — no per-group 64-wide ops (the
v1 bottleneck: 1954 ACT ops at ~518 ns).  All matmul operands are bf16 (v1
paid ~4x for fp32 LDWEIGHTS/MATMUL).  bf16 I/O halves HBM traffic.
"""

import numpy as np
import ml_dtypes

import concourse.bass as bass
import concourse.bacc as bacc
import concourse.tile as tile
from concourse import mybir
from concourse.bass_utils import run_bass_kernel_spmd

F32 = mybir.dt.float32
BF16 = mybir.dt.bfloat16
I32 = mybir.dt.int32
AX = mybir.AluOpType
AF = mybir.ActivationFunctionType

IN_F = 64
OUT_F = 64
EPS = 1e-5
ALPHA = 0.2
N_CORES = 8
N_NODES = 2_000_000

# --- tunables -------------------------------------------------------------
CHUNK_COLS = 8192          # columns per DMA chunk
TILE_COLS = 1024           # columns per compute tile (2 PSUM banks)
IN_BUFS = 3
OUT_BUFS = 3
PSUM_BUFS = 2              # bufs each for y-pool and v-pool (2+2 banks each)
SQ_BUFS = 4
DMA_ENGINE = "sync"
LEAKY_ENGINE = "vector"    # "vector" (stt mult/max) or "scalar" (Prelu)


def build_module(cols, chunk_cols=None, tile_cols=None, passes=1,
                 in_bufs=None, out_bufs=None, psum_bufs=None, sq_bufs=None,
                 leaky_engine=None, dma_engine=None, store_engine=None,
                 variant="full"):
    """Build + compile the Bass module for a per-core shard with `cols`
    columns per block.  cols % tile_cols == 0."""
    chunk_cols = CHUNK_COLS if chunk_cols is None else chunk_cols
    tile_cols = TILE_COLS if tile_cols is None else tile_cols
    in_bufs = IN_BUFS if in_bufs is None else in_bufs
    out_bufs = OUT_BUFS if out_bufs is None else out_bufs
    psum_bufs = PSUM_BUFS if psum_bufs is None else psum_bufs
    sq_bufs = SQ_BUFS if sq_bufs is None else sq_bufs
    leaky_engine = LEAKY_ENGINE if leaky_engine is None else leaky_engine
    dma_engine = DMA_ENGINE if dma_engine is None else dma_engine
    store_engine = dma_engine if store_engine is None else store_engine
    assert cols % tile_cols == 0
    assert chunk_cols % tile_cols == 0
    assert tile_cols % 512 == 0

    nc = bacc.Bacc(
        "TRN2", target_bir_lowering=False, debug=False, enable_asserts=False
    )
    xh = nc.dram_tensor("xh", [128, cols], BF16, kind="ExternalInput").ap()
    wblk = nc.dram_tensor("wblk", [128, 128], BF16, kind="ExternalInput").ap()
    biasw = nc.dram_tensor("biasw", [2, 128], BF16, kind="ExternalInput").ap()
    ones2 = nc.dram_tensor("ones2", [2, 512], BF16, kind="ExternalInput").ap()
    rdiv = nc.dram_tensor("rdiv", [128, 128], BF16, kind="ExternalInput").ap()
    zh = nc.dram_tensor("zh", [128, cols], BF16, kind="ExternalOutput").ap()

    chunks = []
    c0 = 0
    while c0 < cols:
        fc = min(chunk_cols, cols - c0)
        chunks.append((c0, fc))
        c0 += fc

    with tile.TileContext(nc) as tc:
        with (
            tc.tile_pool(name="const", bufs=1) as constp,
            tc.tile_pool(name="inp", bufs=in_bufs) as inp,
            tc.tile_pool(name="outp", bufs=out_bufs) as outp,
            tc.tile_pool(name="psumy", bufs=psum_bufs, space="PSUM") as psumy,
            tc.tile_pool(name="psumv", bufs=psum_bufs, space="PSUM") as psumv,
            tc.tile_pool(name="sqp", bufs=sq_bufs) as sqp,
            tc.tile_pool(name="invp", bufs=sq_bufs) as invp,
            tc.tile_pool(name="lp", bufs=sq_bufs) as lp,
        ):
            wblk_sb = constp.tile([128, 128], BF16, name="wblk_sb")
            nc.sync.dma_start(wblk_sb[:, :], wblk)
            biasw_sb = constp.tile([2, 128], BF16, name="biasw_sb")
            nc.sync.dma_start(biasw_sb[:, :], biasw)
            ones2_sb = constp.tile([2, 512], BF16, name="ones2_sb")
            nc.sync.dma_start(ones2_sb[:, :], ones2)
            rdiv_sb = constp.tile([128, 128], BF16, name="rdiv_sb")
            nc.sync.dma_start(rdiv_sb[:, :], rdiv)
            eps_sb = constp.tile([128, 1], F32, name="eps_sb")
            nc.gpsimd.memset(eps_sb[:, :], float(EPS))

            for ci, (c0, fc) in enumerate(chunks * passes):
                xin = inp.tile([128, chunk_cols], BF16, name="xin", tag="xin")
                getattr(nc, dma_engine).dma_start(
                    xin[:, 0:fc], xh[:, c0 : c0 + fc]
                )
                zout = outp.tile([128, chunk_cols], BF16, name="zout",
                                 tag="zout")

                if variant == "memcpy":
                    getattr(nc, store_engine).dma_start(
                        zh[:, c0 : c0 + fc], xin[:, 0:fc]
                    )
                    continue

                for t0 in range(0, fc, tile_cols):
                    tcw = min(tile_cols, fc - t0)
                    nh = tcw // 512
                    y = psumy.tile([128, tile_cols], F32, name="y", tag="y")
                    for h in range(nh):
                        ys = y[:, h * 512 : (h + 1) * 512]
                        # bias first: start=True clears the bank, then the
                        # main matmul accumulates on top.
                        nc.tensor.matmul(
                            ys, biasw_sb[:, :], ones2_sb[:, :],
                            start=True, stop=False, skip_group_check=True,
                        )
                        nc.tensor.matmul(
                            ys, wblk_sb[:, :],
                            xin[:, t0 + h * 512 : t0 + (h + 1) * 512],
                            start=False, stop=True, skip_group_check=True,
                        )
                    if variant == "matmul_only":
                        nc.vector.tensor_copy(
                            zout[:, t0 : t0 + tcw], y[:, 0:tcw]
                        )
                        continue

                    sq = sqp.tile([128, tile_cols], BF16, name="sq", tag="sq")
                    nc.scalar.activation(sq[:, 0:tcw], y[:, 0:tcw], AF.Square)

                    v = psumv.tile([128, tile_cols], F32, name="v", tag="v")
                    for h in range(nh):
                        nc.tensor.matmul(
                            v[:, h * 512 : (h + 1) * 512],
                            rdiv_sb[:, :],
                            sq[:, h * 512 : (h + 1) * 512],
                            start=True, stop=True, skip_group_check=True,
                        )
                    inv = invp.tile([128, tile_cols], BF16, name="inv",
                                    tag="inv")
                    nc.scalar.activation(
                        inv[:, 0:tcw], v[:, 0:tcw],
                        AF.Abs_reciprocal_sqrt, bias=eps_sb[:, 0:1], scale=1.0,
                    )

                    if leaky_engine == "scalar":
                        # l = Lrelu(y) on ACT (single PSUM input), then
                        # z = l * inv on DVE (bf16 2x mode).
                        l = lp.tile([128, tile_cols], BF16, name="l", tag="l")
                        nc.scalar.activation(
                            l[:, 0:tcw], y[:, 0:tcw], AF.Prelu,
                            bias=0.0, scale=1.0, alpha=ALPHA,
                        )
                        nc.vector.tensor_tensor(
                            zout[:, t0 : t0 + tcw], l[:, 0:tcw],
                            inv[:, 0:tcw], op=AX.mult,
                        )
                    else:
                        # m = y * inv (one PSUM + one SBUF input), then
                        # z = max(alpha*m, m) on the SBUF result.
                        m = lp.tile([128, tile_cols], BF16, name="m", tag="m")
                        nc.vector.tensor_tensor(
                            m[:, 0:tcw], y[:, 0:tcw], inv[:, 0:tcw],
                            op=AX.mult,
                        )
                        nc.vector.scalar_tensor_tensor(
                            zout[:, t0 : t0 + tcw], m[:, 0:tcw], ALPHA,
                            m[:, 0:tcw], op0=AX.mult, op1=AX.max,
                        )

                getattr(nc, store_engine).dma_start(
                    zh[:, c0 : c0 + fc], zout[:, 0:fc]
                )

    nc.compile()
    return nc


# ---------------------------------------------------------------------------
# host-side packing / unpacking
# ---------------------------------------------------------------------------

def _pack_core(shard, cols):
    """[rows, 64] f32 -> xh [128, cols] bf16 (two stacked feature-major
    blocks): xh[b*64+f, c] = shard[b*half + c, f] (zero-padded)."""
    rows = shard.shape[0]
    assert rows % 2 == 0
    half = rows // 2
    xh = np.zeros((128, cols), dtype=ml_dtypes.bfloat16)
    xh[:64, :half] = shard[:half].T.astype(ml_dtypes.bfloat16)
    xh[64:, : rows - half] = shard[half:].T.astype(ml_dtypes.bfloat16)
    return xh


def _unpack_core(zh, cols, rows):
    """zh [128, cols] bf16 -> [rows, 64] f32; inverse of _pack_core."""
    half = rows // 2
    z = np.empty((rows, OUT_F), dtype=np.float32)
    z[:half] = zh[:64, :half].T.astype(np.float32)
    z[half:] = zh[64:, : rows - half].T.astype(np.float32)
    return z


def _make_weights(W, b):
    Wt = W.astype(np.float64).T  # [in_f, out_f]
    Wc = (Wt - Wt.mean(axis=1, keepdims=True)).astype(np.float32)
    wblk = np.zeros((128, 128), dtype=ml_dtypes.bfloat16)
    wblk[:64, :64] = Wc.astype(ml_dtypes.bfloat16)
    wblk[64:, 64:] = Wc.astype(ml_dtypes.bfloat16)
    bc = (b.astype(np.float64) - b.astype(np.float64).mean()).astype(np.float32)
    bc_hi = bc.astype(ml_dtypes.bfloat16)
    bc_lo = (bc - bc_hi.astype(np.float32)).astype(ml_dtypes.bfloat16)
    biasw = np.zeros((2, 128), dtype=ml_dtypes.bfloat16)
    biasw[0] = np.tile(bc_hi, 2)
    biasw[1] = np.tile(bc_lo, 2)
    ones2 = np.ones((2, 512), dtype=ml_dtypes.bfloat16)
    rdiv = np.zeros((128, 128), dtype=ml_dtypes.bfloat16)
    rdiv[:64, :64] = np.float32(1.0 / 64.0)
    rdiv[64:, 64:] = np.float32(1.0 / 64.0)
    return wblk, biasw, ones2, rdiv


_NC_CACHE = {}


def _get_module(cols):
    key = (cols, CHUNK_COLS, TILE_COLS)
    if key not in _NC_CACHE:
        _NC_CACHE[key] = build_module(cols)
    return _NC_CACHE[key]


def _host_reference(input_x, W, b, gamma, beta):
    y = input_x.astype(np.float32) @ W.T.astype(np.float32) + b
    mu = y.mean(axis=-1, keepdims=True)
    var = np.square(y - mu).mean(axis=-1, keepdims=True)
    y = (y - mu) / np.sqrt(var + EPS) * gamma + beta
    return np.where(y >= 0, y, np.float32(ALPHA) * y).astype(np.float32)


def _make_in_maps(input_x, W, b):
    n = input_x.shape[0]
    per_core = (n + N_CORES - 1) // N_CORES
    per_core += (-per_core) % 2
    half = per_core // 2
    cols = ((half + TILE_COLS - 1) // TILE_COLS) * TILE_COLS
    wblk, biasw, ones2, rdiv = _make_weights(W, b)
    in_maps = []
    shards = []
    for i in range(N_CORES):
        lo = min(i * per_core, n)
        hi = min(lo + per_core, n)
        shard = input_x[lo:hi]
        if shard.shape[0] < per_core:
            shard = np.concatenate(
                [shard, np.zeros((per_core - shard.shape[0], IN_F), np.float32)]
            )
        shards.append((lo, hi))
        in_maps.append(
            {"xh": _pack_core(shard, cols), "wblk": wblk, "biasw": biasw,
             "ones2": ones2, "rdiv": rdiv}
        )
    return in_maps, shards, cols, per_core


def make_timed_runner(inputs, warmup=2):
    """Build a persistent sharded-jit over the 8 cores with device-resident
    inputs; returns a callable(iters) -> mean wall seconds per execution."""
    import time
    import jax
    from jax.sharding import Mesh, PartitionSpec, NamedSharding
    from jax.experimental.shard_map import shard_map
    from concourse import bass2jax, mybir as _mb

    bass2jax.install_neuronx_cc_hook()
    input_x = np.asarray(inputs["input_x"], dtype=np.float32)
    W = np.asarray(inputs["W"], dtype=np.float32)
    b = np.asarray(inputs["b"], dtype=np.float32)
    in_maps, shards, cols, per_core = _make_in_maps(input_x, W, b)
    nc = _get_module(cols)

    partition_name = (
        nc.partition_id_tensor.name if nc.partition_id_tensor else None
    )
    in_names, out_names, out_avals, zero_outs = [], [], [], []
    for alloc in nc.m.functions[0].allocations:
        if not isinstance(alloc, _mb.MemoryLocationSet):
            continue
        name = alloc.memorylocations[0].name
        if alloc.kind == "ExternalInput":
            if name != partition_name:
                in_names.append(name)
        elif alloc.kind == "ExternalOutput":
            out_names.append(name)
            shape = tuple(alloc.tensor_shape)
            dtype = _mb.dt.np(alloc.dtype)
            out_avals.append(jax.core.ShapedArray(shape, dtype))
            zero_outs.append(np.zeros(shape, dtype))
    n_params = len(in_names)
    all_names = in_names + out_names
    if partition_name is not None:
        all_names = all_names + [partition_name]

    def _body(*args):
        operands = list(args)
        if partition_name is not None:
            operands.append(bass2jax.partition_id_tensor())
        outs = bass2jax._bass_exec_p.bind(
            *operands,
            out_avals=tuple(out_avals),
            in_names=tuple(all_names),
            out_names=tuple(out_names),
            lowering_input_output_aliases=(),
            sim_require_finite=True,
            sim_require_nnan=True,
            nc=nc,
        )
        return tuple(outs)

    devices = jax.devices()[:N_CORES]
    mesh = Mesh(np.asarray(devices), ("core",))
    spec = PartitionSpec("core")
    sharded = jax.jit(
        shard_map(
            _body, mesh=mesh,
            in_specs=(spec,) * (n_params + len(out_names)),
            out_specs=(spec,) * len(out_names),
            check_rep=False,
        ),
        keep_unused=True,
    )
    sh = NamedSharding(mesh, spec)
    dev_args = [
        jax.device_put(
            np.concatenate([in_maps[c][nm] for c in range(N_CORES)], axis=0), sh
        )
        for nm in in_names
    ] + [
        jax.device_put(
            np.zeros((N_CORES * z.shape[0], *z.shape[1:]), z.dtype), sh
        )
        for z in zero_outs
    ]

    def run(iters=5):
        for _ in range(warmup):
            r = sharded(*dev_args)
            jax.block_until_ready(r)
        t0 = time.perf_counter()
        for _ in range(iters):
            r = sharded(*dev_args)
        jax.block_until_ready(r)
        return (time.perf_counter() - t0) / iters

    return run


def kernel(input_x, W, b, gamma, beta, batch=None, **_unused):
    input_x = np.asarray(input_x, dtype=np.float32)
    W = np.asarray(W, dtype=np.float32)
    b = np.asarray(b, dtype=np.float32)
    gamma = np.asarray(gamma, dtype=np.float32)
    beta = np.asarray(beta, dtype=np.float32)

    if not (np.all(gamma == 1.0) and np.all(beta == 0.0)):
        return _host_reference(input_x, W, b, gamma, beta)

    n = input_x.shape[0]
    in_maps, shards, cols, per_core = _make_in_maps(input_x, W, b)
    nc = _get_module(cols)
    res = run_bass_kernel_spmd(nc, in_maps, core_ids=list(range(N_CORES)))

    out = np.empty((n, OUT_F), dtype=np.float32)
    for i, (lo, hi) in enumerate(shards):
        zh = np.asarray(res.results[i]["zh"])
        z = _unpack_core(zh, cols, per_core)
        out[lo:hi] = z[: hi - lo]
    return out


# revision 12
# speedup vs baseline: 5.6313x; 5.2471x over previous
"""Trainium2 Bass kernel: row-wise Linear(64->64) + LayerNorm + LeakyReLU(0.2).

Math: out = leaky_relu(layernorm(x @ W.T + b)), row-independent; `batch` does
not affect the computation (layernorm is per-row).

v2 design — feature-major layout, bf16 data path, full-width ops only:

  - Host packs each core's row shard [Nc, 64] into xh [128, cols] bf16:
    partition p = (block b in {0,1})*64 + in-feature f; column c = node index
    within the block.  Two node-blocks stack on the partition dim so every
    DMA / matmul / DVE / ACT op uses all 128 partitions.
  - Weights are centered on host (Wc = W.T - rowmean over out-features,
    bc = b - mean(b)) so the matmul directly yields y = out-centered rows:
    LayerNorm's mean subtraction is free.
  - Per 512-col PSUM bank: y = Wblk.T @ x (block-diag Wc, bf16) accumulated
    with a K=2 bias matmul (bc as bf16 hi+lo rows, ones rhs).
  - ACT Square: sq = y^2 (PSUM->SBUF bf16), one full-width op.
  - PE: v = Rdiv.T @ sq with Rdiv = block-diag ones/64 -> v[q, n] = var of
    node n's block, replicated across that block's 64 partitions.
  - ACT Abs_reciprocal_sqrt: inv = (var + eps)^-1/2 (PSUM->SBUF bf16).
  - DVE: l = max(alpha*y, y) (leaky first; valid since inv > 0 commutes),
    then z = l * inv (bf16 tensor_tensor, 2x mode).
  - z streams out as bf16; host unpacks/casts to fp32.

All elementwise work is FD>=512 full-width — no per-group 64-wide ops (the
v1 bottleneck: 1954 ACT ops at ~518 ns).  All matmul operands are bf16 (v1
paid ~4x for fp32 LDWEIGHTS/MATMUL).  bf16 I/O halves HBM traffic.
"""

import numpy as np
import ml_dtypes

import concourse.bass as bass
import concourse.bacc as bacc
import concourse.tile as tile
from concourse import mybir
from concourse.bass_utils import run_bass_kernel_spmd

F32 = mybir.dt.float32
BF16 = mybir.dt.bfloat16
I32 = mybir.dt.int32
AX = mybir.AluOpType
AF = mybir.ActivationFunctionType

IN_F = 64
OUT_F = 64
EPS = 1e-5
ALPHA = 0.2
N_CORES = 8
N_NODES = 2_000_000

# --- tunables -------------------------------------------------------------
CHUNK_COLS = 8192          # columns per DMA chunk
TILE_COLS = 1024           # columns per compute tile (2 PSUM banks)
IN_BUFS = 3
OUT_BUFS = 3
PSUM_BUFS = 2              # bufs each for y-pool and v-pool (2+2 banks each)
SQ_BUFS = 4
DMA_ENGINE = "sync"
LEAKY_ENGINE = "vector"    # "vector" (stt mult/max) or "scalar" (Prelu)


def build_module(cols, chunk_cols=None, tile_cols=None, passes=1,
                 in_bufs=None, out_bufs=None, psum_bufs=None, sq_bufs=None,
                 leaky_engine=None, dma_engine=None, store_engine=None,
                 variant="full"):
    """Build + compile the Bass module for a per-core shard with `cols`
    columns per block.  cols % tile_cols == 0."""
    chunk_cols = CHUNK_COLS if chunk_cols is None else chunk_cols
    tile_cols = TILE_COLS if tile_cols is None else tile_cols
    in_bufs = IN_BUFS if in_bufs is None else in_bufs
    out_bufs = OUT_BUFS if out_bufs is None else out_bufs
    psum_bufs = PSUM_BUFS if psum_bufs is None else psum_bufs
    sq_bufs = SQ_BUFS if sq_bufs is None else sq_bufs
    leaky_engine = LEAKY_ENGINE if leaky_engine is None else leaky_engine
    dma_engine = DMA_ENGINE if dma_engine is None else dma_engine
    store_engine = dma_engine if store_engine is None else store_engine
    assert cols % tile_cols == 0
    assert chunk_cols % tile_cols == 0
    assert tile_cols % 512 == 0

    nc = bacc.Bacc(
        "TRN2", target_bir_lowering=False, debug=False, enable_asserts=False
    )
    xh = nc.dram_tensor("xh", [128, cols], BF16, kind="ExternalInput").ap()
    wblk = nc.dram_tensor("wblk", [128, 128], BF16, kind="ExternalInput").ap()
    bcol = nc.dram_tensor("bcol", [128, 1], F32, kind="ExternalInput").ap()
    rdiv = nc.dram_tensor("rdiv", [128, 128], BF16, kind="ExternalInput").ap()
    zh = nc.dram_tensor("zh", [128, cols], BF16, kind="ExternalOutput").ap()

    chunks = []
    c0 = 0
    while c0 < cols:
        fc = min(chunk_cols, cols - c0)
        chunks.append((c0, fc))
        c0 += fc

    with tile.TileContext(nc) as tc:
        with (
            tc.tile_pool(name="const", bufs=1) as constp,
            tc.tile_pool(name="inp", bufs=in_bufs) as inp,
            tc.tile_pool(name="outp", bufs=out_bufs) as outp,
            tc.tile_pool(name="psumy", bufs=psum_bufs, space="PSUM") as psumy,
            tc.tile_pool(name="psumv", bufs=psum_bufs, space="PSUM") as psumv,
            tc.tile_pool(name="sqp", bufs=sq_bufs) as sqp,
            tc.tile_pool(name="invp", bufs=sq_bufs) as invp,
            tc.tile_pool(name="lp", bufs=sq_bufs) as lp,
        ):
            wblk_sb = constp.tile([128, 128], BF16, name="wblk_sb")
            nc.sync.dma_start(wblk_sb[:, :], wblk)
            bcol_sb = constp.tile([128, 1], F32, name="bcol_sb")
            nc.sync.dma_start(bcol_sb[:, :], bcol)
            rdiv_sb = constp.tile([128, 128], BF16, name="rdiv_sb")
            nc.sync.dma_start(rdiv_sb[:, :], rdiv)
            eps_sb = constp.tile([128, 1], F32, name="eps_sb")
            nc.gpsimd.memset(eps_sb[:, :], float(EPS))

            for ci, (c0, fc) in enumerate(chunks * passes):
                xin = inp.tile([128, chunk_cols], BF16, name="xin", tag="xin")
                getattr(nc, dma_engine).dma_start(
                    xin[:, 0:fc], xh[:, c0 : c0 + fc]
                )
                zout = outp.tile([128, chunk_cols], BF16, name="zout",
                                 tag="zout")

                if variant == "memcpy":
                    getattr(nc, store_engine).dma_start(
                        zh[:, c0 : c0 + fc], xin[:, 0:fc]
                    )
                    continue

                for t0 in range(0, fc, tile_cols):
                    tcw = min(tile_cols, fc - t0)
                    nh = tcw // 512
                    y = psumy.tile([128, tile_cols], F32, name="y", tag="y")
                    for h in range(nh):
                        nc.tensor.matmul(
                            y[:, h * 512 : (h + 1) * 512], wblk_sb[:, :],
                            xin[:, t0 + h * 512 : t0 + (h + 1) * 512],
                            start=True, stop=True, skip_group_check=True,
                        )
                    if variant == "matmul_only":
                        nc.vector.tensor_copy(
                            zout[:, t0 : t0 + tcw], y[:, 0:tcw]
                        )
                        continue

                    # sq = (y + b)^2 — the linear bias rides the ACT bias port
                    sq = sqp.tile([128, tile_cols], BF16, name="sq", tag="sq")
                    nc.scalar.activation(
                        sq[:, 0:tcw], y[:, 0:tcw], AF.Square,
                        bias=bcol_sb[:, 0:1], scale=1.0,
                    )

                    v = psumv.tile([128, tile_cols], F32, name="v", tag="v")
                    for h in range(nh):
                        nc.tensor.matmul(
                            v[:, h * 512 : (h + 1) * 512],
                            rdiv_sb[:, :],
                            sq[:, h * 512 : (h + 1) * 512],
                            start=True, stop=True, skip_group_check=True,
                        )
                    inv = invp.tile([128, tile_cols], BF16, name="inv",
                                    tag="inv")
                    nc.scalar.activation(
                        inv[:, 0:tcw], v[:, 0:tcw],
                        AF.Abs_reciprocal_sqrt, bias=eps_sb[:, 0:1], scale=1.0,
                    )

                    # m = (y + b) * inv (one PSUM + one SBUF input), then
                    # z = max(alpha*m, m) on the SBUF result.
                    m = lp.tile([128, tile_cols], BF16, name="m", tag="m")
                    nc.vector.scalar_tensor_tensor(
                        m[:, 0:tcw], y[:, 0:tcw], bcol_sb[:, 0:1],
                        inv[:, 0:tcw], op0=AX.add, op1=AX.mult,
                    )
                    if leaky_engine == "gpsimd":
                        nc.gpsimd.scalar_tensor_tensor(
                            zout[:, t0 : t0 + tcw], m[:, 0:tcw], ALPHA,
                            m[:, 0:tcw], op0=AX.mult, op1=AX.max,
                        )
                    else:
                        nc.vector.scalar_tensor_tensor(
                            zout[:, t0 : t0 + tcw], m[:, 0:tcw], ALPHA,
                            m[:, 0:tcw], op0=AX.mult, op1=AX.max,
                        )

                getattr(nc, store_engine).dma_start(
                    zh[:, c0 : c0 + fc], zout[:, 0:fc]
                )

    nc.compile()
    return nc


# ---------------------------------------------------------------------------
# host-side packing / unpacking
# ---------------------------------------------------------------------------

def _pack_core(shard, cols):
    """[rows, 64] f32 -> xh [128, cols] bf16 (two stacked feature-major
    blocks): xh[b*64+f, c] = shard[b*half + c, f] (zero-padded)."""
    rows = shard.shape[0]
    assert rows % 2 == 0
    half = rows // 2
    xh = np.zeros((128, cols), dtype=ml_dtypes.bfloat16)
    xh[:64, :half] = shard[:half].T.astype(ml_dtypes.bfloat16)
    xh[64:, : rows - half] = shard[half:].T.astype(ml_dtypes.bfloat16)
    return xh


def _unpack_core(zh, cols, rows):
    """zh [128, cols] bf16 -> [rows, 64] f32; inverse of _pack_core."""
    half = rows // 2
    z = np.empty((rows, OUT_F), dtype=np.float32)
    z[:half] = zh[:64, :half].T.astype(np.float32)
    z[half:] = zh[64:, : rows - half].T.astype(np.float32)
    return z


def _make_weights(W, b):
    Wt = W.astype(np.float64).T  # [in_f, out_f]
    Wc = (Wt - Wt.mean(axis=1, keepdims=True)).astype(np.float32)
    wblk = np.zeros((128, 128), dtype=ml_dtypes.bfloat16)
    wblk[:64, :64] = Wc.astype(ml_dtypes.bfloat16)
    wblk[64:, 64:] = Wc.astype(ml_dtypes.bfloat16)
    bc = (b.astype(np.float64) - b.astype(np.float64).mean()).astype(np.float32)
    bcol = np.tile(bc, 2).reshape(128, 1).astype(np.float32)
    rdiv = np.zeros((128, 128), dtype=ml_dtypes.bfloat16)
    rdiv[:64, :64] = np.float32(1.0 / 64.0)
    rdiv[64:, 64:] = np.float32(1.0 / 64.0)
    return wblk, bcol, rdiv


_NC_CACHE = {}


def _get_module(cols):
    key = (cols, CHUNK_COLS, TILE_COLS)
    if key not in _NC_CACHE:
        _NC_CACHE[key] = build_module(cols)
    return _NC_CACHE[key]


def _host_reference(input_x, W, b, gamma, beta):
    y = input_x.astype(np.float32) @ W.T.astype(np.float32) + b
    mu = y.mean(axis=-1, keepdims=True)
    var = np.square(y - mu).mean(axis=-1, keepdims=True)
    y = (y - mu) / np.sqrt(var + EPS) * gamma + beta
    return np.where(y >= 0, y, np.float32(ALPHA) * y).astype(np.float32)


def _make_in_maps(input_x, W, b):
    n = input_x.shape[0]
    per_core = (n + N_CORES - 1) // N_CORES
    per_core += (-per_core) % 2
    half = per_core // 2
    cols = ((half + TILE_COLS - 1) // TILE_COLS) * TILE_COLS
    wblk, bcol, rdiv = _make_weights(W, b)
    in_maps = []
    shards = []
    for i in range(N_CORES):
        lo = min(i * per_core, n)
        hi = min(lo + per_core, n)
        shard = input_x[lo:hi]
        if shard.shape[0] < per_core:
            shard = np.concatenate(
                [shard, np.zeros((per_core - shard.shape[0], IN_F), np.float32)]
            )
        shards.append((lo, hi))
        in_maps.append(
            {"xh": _pack_core(shard, cols), "wblk": wblk, "bcol": bcol,
             "rdiv": rdiv}
        )
    return in_maps, shards, cols, per_core


def make_timed_runner(inputs, warmup=2):
    """Build a persistent sharded-jit over the 8 cores with device-resident
    inputs; returns a callable(iters) -> mean wall seconds per execution."""
    import time
    import jax
    from jax.sharding import Mesh, PartitionSpec, NamedSharding
    from jax.experimental.shard_map import shard_map
    from concourse import bass2jax, mybir as _mb

    bass2jax.install_neuronx_cc_hook()
    input_x = np.asarray(inputs["input_x"], dtype=np.float32)
    W = np.asarray(inputs["W"], dtype=np.float32)
    b = np.asarray(inputs["b"], dtype=np.float32)
    in_maps, shards, cols, per_core = _make_in_maps(input_x, W, b)
    nc = _get_module(cols)

    partition_name = (
        nc.partition_id_tensor.name if nc.partition_id_tensor else None
    )
    in_names, out_names, out_avals, zero_outs = [], [], [], []
    for alloc in nc.m.functions[0].allocations:
        if not isinstance(alloc, _mb.MemoryLocationSet):
            continue
        name = alloc.memorylocations[0].name
        if alloc.kind == "ExternalInput":
            if name != partition_name:
                in_names.append(name)
        elif alloc.kind == "ExternalOutput":
            out_names.append(name)
            shape = tuple(alloc.tensor_shape)
            dtype = _mb.dt.np(alloc.dtype)
            out_avals.append(jax.core.ShapedArray(shape, dtype))
            zero_outs.append(np.zeros(shape, dtype))
    n_params = len(in_names)
    all_names = in_names + out_names
    if partition_name is not None:
        all_names = all_names + [partition_name]

    def _body(*args):
        operands = list(args)
        if partition_name is not None:
            operands.append(bass2jax.partition_id_tensor())
        outs = bass2jax._bass_exec_p.bind(
            *operands,
            out_avals=tuple(out_avals),
            in_names=tuple(all_names),
            out_names=tuple(out_names),
            lowering_input_output_aliases=(),
            sim_require_finite=True,
            sim_require_nnan=True,
            nc=nc,
        )
        return tuple(outs)

    devices = jax.devices()[:N_CORES]
    mesh = Mesh(np.asarray(devices), ("core",))
    spec = PartitionSpec("core")
    sharded = jax.jit(
        shard_map(
            _body, mesh=mesh,
            in_specs=(spec,) * (n_params + len(out_names)),
            out_specs=(spec,) * len(out_names),
            check_rep=False,
        ),
        keep_unused=True,
    )
    sh = NamedSharding(mesh, spec)
    dev_args = [
        jax.device_put(
            np.concatenate([in_maps[c][nm] for c in range(N_CORES)], axis=0), sh
        )
        for nm in in_names
    ] + [
        jax.device_put(
            np.zeros((N_CORES * z.shape[0], *z.shape[1:]), z.dtype), sh
        )
        for z in zero_outs
    ]

    def run(iters=5):
        for _ in range(warmup):
            r = sharded(*dev_args)
            jax.block_until_ready(r)
        t0 = time.perf_counter()
        for _ in range(iters):
            r = sharded(*dev_args)
        jax.block_until_ready(r)
        return (time.perf_counter() - t0) / iters

    return run


def kernel(input_x, W, b, gamma, beta, batch=None, **_unused):
    input_x = np.asarray(input_x, dtype=np.float32)
    W = np.asarray(W, dtype=np.float32)
    b = np.asarray(b, dtype=np.float32)
    gamma = np.asarray(gamma, dtype=np.float32)
    beta = np.asarray(beta, dtype=np.float32)

    if not (np.all(gamma == 1.0) and np.all(beta == 0.0)):
        return _host_reference(input_x, W, b, gamma, beta)

    n = input_x.shape[0]
    in_maps, shards, cols, per_core = _make_in_maps(input_x, W, b)
    nc = _get_module(cols)
    res = run_bass_kernel_spmd(nc, in_maps, core_ids=list(range(N_CORES)))

    out = np.empty((n, OUT_F), dtype=np.float32)
    for i, (lo, hi) in enumerate(shards):
        zh = np.asarray(res.results[i]["zh"])
        z = _unpack_core(zh, cols, per_core)
        out[lo:hi] = z[: hi - lo]
    return out


# revision 13
# speedup vs baseline: 12.4752x; 2.2153x over previous
"""Trainium2 Bass kernel: row-wise Linear(64->64) + LayerNorm + LeakyReLU(0.2).

Math: out = leaky_relu(layernorm(x @ W.T + b)), row-independent; `batch` does
not affect the computation (layernorm is per-row).

v2 design — feature-major layout, bf16 data path, full-width ops only:

  - Host packs each core's row shard [Nc, 64] into xh [128, cols] bf16:
    partition p = (block b in {0,1})*64 + in-feature f; column c = node index
    within the block.  Two node-blocks stack on the partition dim so every
    DMA / matmul / DVE / ACT op uses all 128 partitions.
  - Weights are centered on host (Wc = W.T - rowmean over out-features,
    bc = b - mean(b)) so the matmul directly yields y = out-centered rows:
    LayerNorm's mean subtraction is free.
  - Per 512-col PSUM bank: y = Wblk.T @ x (block-diag Wc, bf16) accumulated
    with a K=2 bias matmul (bc as bf16 hi+lo rows, ones rhs).
  - ACT Square: sq = y^2 (PSUM->SBUF bf16), one full-width op.
  - PE: v = Rdiv.T @ sq with Rdiv = block-diag ones/64 -> v[q, n] = var of
    node n's block, replicated across that block's 64 partitions.
  - ACT Abs_reciprocal_sqrt: inv = (var + eps)^-1/2 (PSUM->SBUF bf16).
  - DVE: l = max(alpha*y, y) (leaky first; valid since inv > 0 commutes),
    then z = l * inv (bf16 tensor_tensor, 2x mode).
  - z streams out as bf16; host unpacks/casts to fp32.

All elementwise work is FD>=512 full-width — no per-group 64-wide ops (the
v1 bottleneck: 1954 ACT ops at ~518 ns).  All matmul operands are bf16 (v1
paid ~4x for fp32 LDWEIGHTS/MATMUL).  bf16 I/O halves HBM traffic.
"""

import numpy as np
import ml_dtypes

import concourse.bass as bass
import concourse.bacc as bacc
import concourse.tile as tile
from concourse import mybir
from concourse.bass_utils import run_bass_kernel_spmd

F32 = mybir.dt.float32
BF16 = mybir.dt.bfloat16
I32 = mybir.dt.int32
AX = mybir.AluOpType
AF = mybir.ActivationFunctionType

IN_F = 64
OUT_F = 64
EPS = 1e-5
ALPHA = 0.2
N_CORES = 8
N_NODES = 2_000_000

# --- tunables -------------------------------------------------------------
CHUNK_COLS = 8192          # columns per DMA chunk
TILE_COLS = 512            # columns per compute tile (1 PSUM bank)
IN_BUFS = 3
OUT_BUFS = 3
PSUM_BUFS = 4              # bufs each for y-pool and v-pool (4+4 banks)
SQ_BUFS = 4
DMA_ENGINE = "sync"
LEAKY_ENGINE = "vector"    # "vector" (stt mult/max) or "scalar" (Prelu)


def build_module(cols, chunk_cols=None, tile_cols=None, passes=1,
                 in_bufs=None, out_bufs=None, psum_bufs=None, sq_bufs=None,
                 leaky_engine=None, dma_engine=None, store_engine=None,
                 variant="full"):
    """Build + compile the Bass module for a per-core shard with `cols`
    columns per block.  cols % tile_cols == 0."""
    chunk_cols = CHUNK_COLS if chunk_cols is None else chunk_cols
    tile_cols = TILE_COLS if tile_cols is None else tile_cols
    in_bufs = IN_BUFS if in_bufs is None else in_bufs
    out_bufs = OUT_BUFS if out_bufs is None else out_bufs
    psum_bufs = PSUM_BUFS if psum_bufs is None else psum_bufs
    sq_bufs = SQ_BUFS if sq_bufs is None else sq_bufs
    leaky_engine = LEAKY_ENGINE if leaky_engine is None else leaky_engine
    dma_engine = DMA_ENGINE if dma_engine is None else dma_engine
    store_engine = dma_engine if store_engine is None else store_engine
    assert cols % tile_cols == 0
    assert chunk_cols % tile_cols == 0
    assert tile_cols % 512 == 0

    nc = bacc.Bacc(
        "TRN2", target_bir_lowering=False, debug=False, enable_asserts=False
    )
    xh = nc.dram_tensor("xh", [128, cols], BF16, kind="ExternalInput").ap()
    wblk = nc.dram_tensor("wblk", [128, 128], BF16, kind="ExternalInput").ap()
    bcol = nc.dram_tensor("bcol", [128, 1], F32, kind="ExternalInput").ap()
    rdiv = nc.dram_tensor("rdiv", [128, 128], BF16, kind="ExternalInput").ap()
    zh = nc.dram_tensor("zh", [128, cols], BF16, kind="ExternalOutput").ap()

    chunks = []
    c0 = 0
    while c0 < cols:
        fc = min(chunk_cols, cols - c0)
        chunks.append((c0, fc))
        c0 += fc

    with tile.TileContext(nc) as tc:
        with (
            tc.tile_pool(name="const", bufs=1) as constp,
            tc.tile_pool(name="inp", bufs=in_bufs) as inp,
            tc.tile_pool(name="outp", bufs=out_bufs) as outp,
            tc.tile_pool(name="psumy", bufs=psum_bufs, space="PSUM") as psumy,
            tc.tile_pool(name="psumv", bufs=psum_bufs, space="PSUM") as psumv,
            tc.tile_pool(name="sqp", bufs=sq_bufs) as sqp,
            tc.tile_pool(name="invp", bufs=sq_bufs) as invp,
            tc.tile_pool(name="lp", bufs=sq_bufs) as lp,
        ):
            wblk_sb = constp.tile([128, 128], BF16, name="wblk_sb")
            nc.sync.dma_start(wblk_sb[:, :], wblk)
            bcol_sb = constp.tile([128, 1], F32, name="bcol_sb")
            nc.sync.dma_start(bcol_sb[:, :], bcol)
            rdiv_sb = constp.tile([128, 128], BF16, name="rdiv_sb")
            nc.sync.dma_start(rdiv_sb[:, :], rdiv)
            eps_sb = constp.tile([128, 1], F32, name="eps_sb")
            nc.gpsimd.memset(eps_sb[:, :], float(EPS))

            for ci, (c0, fc) in enumerate(chunks * passes):
                xin = inp.tile([128, chunk_cols], BF16, name="xin", tag="xin")
                getattr(nc, dma_engine).dma_start(
                    xin[:, 0:fc], xh[:, c0 : c0 + fc]
                )
                zout = outp.tile([128, chunk_cols], BF16, name="zout",
                                 tag="zout")

                if variant == "memcpy":
                    getattr(nc, store_engine).dma_start(
                        zh[:, c0 : c0 + fc], xin[:, 0:fc]
                    )
                    continue

                for t0 in range(0, fc, tile_cols):
                    tcw = min(tile_cols, fc - t0)
                    nh = tcw // 512
                    y = psumy.tile([128, tile_cols], F32, name="y", tag="y")
                    for h in range(nh):
                        nc.tensor.matmul(
                            y[:, h * 512 : (h + 1) * 512], wblk_sb[:, :],
                            xin[:, t0 + h * 512 : t0 + (h + 1) * 512],
                            start=True, stop=True, skip_group_check=True,
                        )
                    if variant == "matmul_only":
                        nc.vector.tensor_copy(
                            zout[:, t0 : t0 + tcw], y[:, 0:tcw]
                        )
                        continue

                    # sq = (y + b)^2 — the linear bias rides the ACT bias port
                    sq = sqp.tile([128, tile_cols], BF16, name="sq", tag="sq")
                    nc.scalar.activation(
                        sq[:, 0:tcw], y[:, 0:tcw], AF.Square,
                        bias=bcol_sb[:, 0:1], scale=1.0,
                    )

                    v = psumv.tile([128, tile_cols], F32, name="v", tag="v")
                    for h in range(nh):
                        nc.tensor.matmul(
                            v[:, h * 512 : (h + 1) * 512],
                            rdiv_sb[:, :],
                            sq[:, h * 512 : (h + 1) * 512],
                            start=True, stop=True, skip_group_check=True,
                        )
                    inv = invp.tile([128, tile_cols], BF16, name="inv",
                                    tag="inv")
                    nc.scalar.activation(
                        inv[:, 0:tcw], v[:, 0:tcw],
                        AF.Abs_reciprocal_sqrt, bias=eps_sb[:, 0:1], scale=1.0,
                    )

                    # m = (y + b) * inv (one PSUM + one SBUF input), then
                    # z = max(alpha*m, m) on the SBUF result.
                    m = lp.tile([128, tile_cols], BF16, name="m", tag="m")
                    nc.vector.scalar_tensor_tensor(
                        m[:, 0:tcw], y[:, 0:tcw], bcol_sb[:, 0:1],
                        inv[:, 0:tcw], op0=AX.add, op1=AX.mult,
                    )
                    if leaky_engine == "gpsimd":
                        nc.gpsimd.scalar_tensor_tensor(
                            zout[:, t0 : t0 + tcw], m[:, 0:tcw], ALPHA,
                            m[:, 0:tcw], op0=AX.mult, op1=AX.max,
                        )
                    else:
                        nc.vector.scalar_tensor_tensor(
                            zout[:, t0 : t0 + tcw], m[:, 0:tcw], ALPHA,
                            m[:, 0:tcw], op0=AX.mult, op1=AX.max,
                        )

                getattr(nc, store_engine).dma_start(
                    zh[:, c0 : c0 + fc], zout[:, 0:fc]
                )

    nc.compile()
    return nc


# ---------------------------------------------------------------------------
# host-side packing / unpacking
# ---------------------------------------------------------------------------

def _pack_core(shard, cols):
    """[rows, 64] f32 -> xh [128, cols] bf16 (two stacked feature-major
    blocks): xh[b*64+f, c] = shard[b*half + c, f] (zero-padded)."""
    rows = shard.shape[0]
    assert rows % 2 == 0
    half = rows // 2
    xh = np.zeros((128, cols), dtype=ml_dtypes.bfloat16)
    xh[:64, :half] = shard[:half].T.astype(ml_dtypes.bfloat16)
    xh[64:, : rows - half] = shard[half:].T.astype(ml_dtypes.bfloat16)
    return xh


def _unpack_core(zh, cols, rows):
    """zh [128, cols] bf16 -> [rows, 64] f32; inverse of _pack_core."""
    half = rows // 2
    z = np.empty((rows, OUT_F), dtype=np.float32)
    z[:half] = zh[:64, :half].T.astype(np.float32)
    z[half:] = zh[64:, : rows - half].T.astype(np.float32)
    return z


def _make_weights(W, b):
    Wt = W.astype(np.float64).T  # [in_f, out_f]
    Wc = (Wt - Wt.mean(axis=1, keepdims=True)).astype(np.float32)
    wblk = np.zeros((128, 128), dtype=ml_dtypes.bfloat16)
    wblk[:64, :64] = Wc.astype(ml_dtypes.bfloat16)
    wblk[64:, 64:] = Wc.astype(ml_dtypes.bfloat16)
    bc = (b.astype(np.float64) - b.astype(np.float64).mean()).astype(np.float32)
    bcol = np.tile(bc, 2).reshape(128, 1).astype(np.float32)
    rdiv = np.zeros((128, 128), dtype=ml_dtypes.bfloat16)
    rdiv[:64, :64] = np.float32(1.0 / 64.0)
    rdiv[64:, 64:] = np.float32(1.0 / 64.0)
    return wblk, bcol, rdiv


_NC_CACHE = {}


def _get_module(cols):
    key = (cols, CHUNK_COLS, TILE_COLS)
    if key not in _NC_CACHE:
        _NC_CACHE[key] = build_module(cols)
    return _NC_CACHE[key]


def _host_reference(input_x, W, b, gamma, beta):
    y = input_x.astype(np.float32) @ W.T.astype(np.float32) + b
    mu = y.mean(axis=-1, keepdims=True)
    var = np.square(y - mu).mean(axis=-1, keepdims=True)
    y = (y - mu) / np.sqrt(var + EPS) * gamma + beta
    return np.where(y >= 0, y, np.float32(ALPHA) * y).astype(np.float32)


def _make_in_maps(input_x, W, b):
    n = input_x.shape[0]
    per_core = (n + N_CORES - 1) // N_CORES
    per_core += (-per_core) % 2
    half = per_core // 2
    cols = ((half + TILE_COLS - 1) // TILE_COLS) * TILE_COLS
    wblk, bcol, rdiv = _make_weights(W, b)
    in_maps = []
    shards = []
    for i in range(N_CORES):
        lo = min(i * per_core, n)
        hi = min(lo + per_core, n)
        shard = input_x[lo:hi]
        if shard.shape[0] < per_core:
            shard = np.concatenate(
                [shard, np.zeros((per_core - shard.shape[0], IN_F), np.float32)]
            )
        shards.append((lo, hi))
        in_maps.append(
            {"xh": _pack_core(shard, cols), "wblk": wblk, "bcol": bcol,
             "rdiv": rdiv}
        )
    return in_maps, shards, cols, per_core


def make_timed_runner(inputs, warmup=2):
    """Build a persistent sharded-jit over the 8 cores with device-resident
    inputs; returns a callable(iters) -> mean wall seconds per execution."""
    import time
    import jax
    from jax.sharding import Mesh, PartitionSpec, NamedSharding
    from jax.experimental.shard_map import shard_map
    from concourse import bass2jax, mybir as _mb

    bass2jax.install_neuronx_cc_hook()
    input_x = np.asarray(inputs["input_x"], dtype=np.float32)
    W = np.asarray(inputs["W"], dtype=np.float32)
    b = np.asarray(inputs["b"], dtype=np.float32)
    in_maps, shards, cols, per_core = _make_in_maps(input_x, W, b)
    nc = _get_module(cols)

    partition_name = (
        nc.partition_id_tensor.name if nc.partition_id_tensor else None
    )
    in_names, out_names, out_avals, zero_outs = [], [], [], []
    for alloc in nc.m.functions[0].allocations:
        if not isinstance(alloc, _mb.MemoryLocationSet):
            continue
        name = alloc.memorylocations[0].name
        if alloc.kind == "ExternalInput":
            if name != partition_name:
                in_names.append(name)
        elif alloc.kind == "ExternalOutput":
            out_names.append(name)
            shape = tuple(alloc.tensor_shape)
            dtype = _mb.dt.np(alloc.dtype)
            out_avals.append(jax.core.ShapedArray(shape, dtype))
            zero_outs.append(np.zeros(shape, dtype))
    n_params = len(in_names)
    all_names = in_names + out_names
    if partition_name is not None:
        all_names = all_names + [partition_name]

    def _body(*args):
        operands = list(args)
        if partition_name is not None:
            operands.append(bass2jax.partition_id_tensor())
        outs = bass2jax._bass_exec_p.bind(
            *operands,
            out_avals=tuple(out_avals),
            in_names=tuple(all_names),
            out_names=tuple(out_names),
            lowering_input_output_aliases=(),
            sim_require_finite=True,
            sim_require_nnan=True,
            nc=nc,
        )
        return tuple(outs)

    devices = jax.devices()[:N_CORES]
    mesh = Mesh(np.asarray(devices), ("core",))
    spec = PartitionSpec("core")
    sharded = jax.jit(
        shard_map(
            _body, mesh=mesh,
            in_specs=(spec,) * (n_params + len(out_names)),
            out_specs=(spec,) * len(out_names),
            check_rep=False,
        ),
        keep_unused=True,
    )
    sh = NamedSharding(mesh, spec)
    dev_args = [
        jax.device_put(
            np.concatenate([in_maps[c][nm] for c in range(N_CORES)], axis=0), sh
        )
        for nm in in_names
    ] + [
        jax.device_put(
            np.zeros((N_CORES * z.shape[0], *z.shape[1:]), z.dtype), sh
        )
        for z in zero_outs
    ]

    def run(iters=5):
        for _ in range(warmup):
            r = sharded(*dev_args)
            jax.block_until_ready(r)
        t0 = time.perf_counter()
        for _ in range(iters):
            r = sharded(*dev_args)
        jax.block_until_ready(r)
        return (time.perf_counter() - t0) / iters

    return run


def kernel(input_x, W, b, gamma, beta, batch=None, **_unused):
    input_x = np.asarray(input_x, dtype=np.float32)
    W = np.asarray(W, dtype=np.float32)
    b = np.asarray(b, dtype=np.float32)
    gamma = np.asarray(gamma, dtype=np.float32)
    beta = np.asarray(beta, dtype=np.float32)

    if not (np.all(gamma == 1.0) and np.all(beta == 0.0)):
        return _host_reference(input_x, W, b, gamma, beta)

    n = input_x.shape[0]
    in_maps, shards, cols, per_core = _make_in_maps(input_x, W, b)
    nc = _get_module(cols)
    res = run_bass_kernel_spmd(nc, in_maps, core_ids=list(range(N_CORES)))

    out = np.empty((n, OUT_F), dtype=np.float32)
    for i, (lo, hi) in enumerate(shards):
        zh = np.asarray(res.results[i]["zh"])
        z = _unpack_core(zh, cols, per_core)
        out[lo:hi] = z[: hi - lo]
    return out
